# revision 42
# baseline (speedup 1.0000x reference)
"""EquivariantUpdate Bass kernel for 8 TRN2 NeuronCores.

Strategy (edge-sharded, per sharding hint):
- Host: shard E=800k edges 8 ways; per core, bucket edges by
  (row>=25000, col>=25000) so dma_gather's int16 indices work against
  half-table views. No sorting needed.
- Device: A = h @ W1a.T, B = h @ W1b.T tables in bf16; dma_gather
  A[row], B[col] over 4 SWDGE queues; per-edge MLP on PE/ACT/DVE;
  trans = (coord_diff*edge_mask) * phi; segment-sum via
  dma_scatter_add with SBUF destination (CCE add into on-chip
  accumulator surfaces). The row-half bucket split keeps scatter
  indices int16: rows<25000 go to surface pair A (idx=row+256),
  rows>=25000 to pair B (idx=row-25000+256); idx group 0 is a dump
  column for pad edges. Parity (bit 7 of idx) splits each pair into
  own/other surfaces per the HW decode. Two rotating surface sets
  break the WAW chain between consecutive scatters; merged on-chip
  at the end and DMA'd out as raw surfaces.
- Host: decode surfaces -> [N,3], sum the 8 per-core partials,
  out = (coord + agg) * node_mask.
  (1/NORM_FACTOR is folded into W3; edge_mask folded into coord_diff.)
"""
import numpy as np
import ml_dtypes

import concourse.bass as bass
import concourse.bacc as bacc
import concourse.mybir as mybir
import concourse.tile as tile
from concourse.bass_utils import run_bass_kernel_spmd
from concourse.masks import make_identity

P = 128
N = 50000
H = 128
E = 800000
NCORES = 8
ECORE = E // NCORES          # 100000
HALF = 24960                 # table split point (multiple of 128, int16-safe)
NLO = HALF                   # nodes in low half
NHI = N - HALF               # 25040 nodes in high half
WLO = NLO // P               # 195 windows (tiled table layout)
WHI = (NHI + P - 1) // P     # 196 windows
NHIP = WHI * P               # 25088 padded rows for the high half
NI = 2048                    # indices per dma_gather / dma_scatter_add
TILES_PER_GI = NI // P       # 16
BCAP = 26624                 # padded edges per bucket (13 * 2048)
GI_PER_B = BCAP // NI        # 13
TB = BCAP // P               # 208 tiles per bucket
NBUCK = 4
TTOT = NBUCK * TB            # 832 tiles per core
G = 99                       # scatter groups per surface (idx<=25255 -> g<=98)
G4 = G * 4                   # f32 words per partition per surface
NSETS = 4                    # rotating surface sets (break WAW chains)

BF16 = mybir.dt.bfloat16
F32 = mybir.dt.float32
F8 = mybir.dt.float8e4
I16 = mybir.dt.int16
I32 = mybir.dt.int32

_nc_cache = {}


def _wrap_idx(idx_flat):
    """int16 indices -> wrapped [16, NI/16] replicated to [128, NI/16]."""
    w = idx_flat.reshape(NI // 16, 16).T.astype(np.int16)
    return np.tile(w, (8, 1))


def _build_program(loop_k=0, ablate=None):
    import os
    ablate = ablate or os.environ.get("KABLATE", "")
    import contextlib
    nc = bacc.Bacc(None, target_bir_lowering=False, num_swdge_queues=4)

    # ---- inputs (per-core values, same shapes everywhere) ----
    hT_t = nc.dram_tensor("hT", [H, N], BF16, kind="ExternalInput")
    w1abT_t = nc.dram_tensor("w1abT", [H, 2 * H], BF16, kind="ExternalInput")
    w1c_t = nc.dram_tensor("w1c", [1, H], BF16, kind="ExternalInput")
    w2T_t = nc.dram_tensor("w2T", [H, H], BF16, kind="ExternalInput")
    w3Ts_t = nc.dram_tensor("w3Ts", [H, 1], BF16, kind="ExternalInput")
    b1_t = nc.dram_tensor("b1c", [H, 1], F32, kind="ExternalInput")
    b2_t = nc.dram_tensor("b2c", [H, 1], F32, kind="ExternalInput")
    # gather indices: per bucket, GI_PER_B instrs x (A then B) wrapped tiles
    idxg_t = nc.dram_tensor("idxg", [NBUCK, GI_PER_B, 2, P, NI // 16], I16,
                            kind="ExternalInput")
    # scatter indices: per (bucket, gi) wrapped token idx into surface pair
    idxsc_t = nc.dram_tensor("idxsc", [NBUCK, GI_PER_B, P, NI // 16], I16,
                             kind="ExternalInput")
    cdm_t = nc.dram_tensor("cdm", [P, TTOT * 4], BF16, kind="ExternalInput")
    attrT_t = nc.dram_tensor("attrT", [NBUCK, 1, BCAP], BF16, kind="ExternalInput")

    aggs_t = nc.dram_tensor("aggs", [P, 4 * G4], F32, kind="ExternalOutput")

    # node tables in tiled layout: node m of a half lives at flat row
    # (m % 128) * W + m // 128, so phase-0 writes are >=1KB per partition
    atab_lo = nc.dram_tensor("atab_lo", [NLO, H], BF16)
    atab_hi = nc.dram_tensor("atab_hi", [NHIP, H], BF16)
    btab_lo = nc.dram_tensor("btab_lo", [NLO, H], BF16)
    btab_hi = nc.dram_tensor("btab_hi", [NHIP, H], BF16)

    NT_N = (N + P - 1) // P  # 391 node tiles
    with tile.TileContext(nc) as tc:
        with (
            tc.tile_pool(name="static", bufs=1) as stp,
            tc.tile_pool(name="p0", bufs=4) as p0p,
            tc.tile_pool(name="p0ps", bufs=1, space="PSUM") as p0ps,
            tc.tile_pool(name="gat", bufs=2) as gap,
            tc.tile_pool(name="blk", bufs=3) as blp,
            tc.tile_pool(name="ps", bufs=2, space="PSUM") as psp,
            tc.tile_pool(name="phips", bufs=2, space="PSUM") as phps,
            tc.tile_pool(name="sc", bufs=8) as scp,
            tc.tile_pool(name="src", bufs=3) as srp,
        ):
            # ---- statics ----
            ident = stp.tile([P, P], F32)
            make_identity(nc, ident[:])
            w1abT = stp.tile([H, 2 * H], BF16)
            nc.sync.dma_start(out=w1abT[:], in_=w1abT_t[:, :])
            w1c = stp.tile([1, H], BF16)
            nc.sync.dma_start(out=w1c[:], in_=w1c_t[:, :])
            w2T = stp.tile([H, H], BF16)
            nc.sync.dma_start(out=w2T[:], in_=w2T_t[:, :])
            w3Ts = stp.tile([H, 1], BF16)
            nc.sync.dma_start(out=w3Ts[:], in_=w3Ts_t[:, :])
            b1 = stp.tile([H, 1], F32)
            nc.sync.dma_start(out=b1[:], in_=b1_t[:, :])
            b2 = stp.tile([H, 1], F32)
            nc.sync.dma_start(out=b2[:], in_=b2_t[:, :])
            cdmR = stp.tile([P, TTOT * 4], BF16)
            nc.sync.dma_start(out=cdmR[:], in_=cdm_t[:, :])
            # index slabs: one DMA each instead of 3 per gather batch
            idxgA = stp.tile([P, NBUCK * GI_PER_B * 2 * (NI // 16)], I16)
            nc.sync.dma_start(
                out=idxgA[:].rearrange("p (n f) -> p n f", n=NBUCK * GI_PER_B * 2),
                in_=idxg_t[:, :, :, :, :].rearrange("n g t p f -> p (n g t) f"))
            idxscA = stp.tile([P, NBUCK * GI_PER_B * (NI // 16)], I16)
            nc.sync.dma_start(
                out=idxscA[:].rearrange("p (n f) -> p n f", n=NBUCK * GI_PER_B),
                in_=idxsc_t[:, :, :, :].rearrange("n g p f -> p (n g) f"))

            # ---- accumulator surfaces: [set][pair a/b][parity 0/1] ----
            surf = [[stp.tile([P, G4], F32, tag=f"surf{s}{i}", name=f"surf{s}{i}")
                     for i in range(4)]
                    for s in range(NSETS)]

            loop_cm = tc.For_i(0, loop_k, 1) if loop_k else contextlib.nullcontext()
            loop_cm.__enter__()
            for s in range(NSETS):
                for i in range(4):
                    nc.vector.memset(surf[s][i][:], 0.0)

            # ---- phase 0: A/B tables (512-node chunks, tiled-layout writes) ----
            NCHUNK = 512
            halves = ((0, NLO, atab_lo, btab_lo), (NLO, NHI, atab_hi, btab_hi))
            if "nophase0" in ablate:
                halves = ()
            for base, cnt, at_t, bt_t in halves:
                atv = at_t[:, :].rearrange("(p w) h -> p w h", p=P)
                btv = bt_t[:, :].rearrange("(p w) h -> p w h", p=P)
                for c0 in range(0, cnt, NCHUNK):
                    cw = min(NCHUNK, cnt - c0)
                    nt = (cw + P - 1) // P
                    ntf = cw // P
                    w0 = c0 // P
                    ht = p0p.tile([H, NCHUNK], BF16, tag="ht")
                    nc.sync.dma_start(out=ht[:, :cw],
                                      in_=hT_t[:, base + c0 : base + c0 + cw])
                    absa = p0p.tile([P, (NCHUNK // P) * H], BF16, tag="absa")
                    absd = p0p.tile([P, (NCHUNK // P) * H], BF16, tag="absd")
                    ab = p0ps.tile([P, (NCHUNK // P) * 2 * H], F32, space="PSUM",
                                   tag="abps")
                    for t in range(nt):
                        w = min(P, cw - t * P)
                        nc.tensor.matmul(ab[:w, t * 2 * H : (t + 1) * 2 * H],
                                         lhsT=ht[:, t * P : t * P + w],
                                         rhs=w1abT[:], start=True, stop=True)
                    abv = ab[:].rearrange("p (t f) -> p t f", f=2 * H)
                    if ntf:
                        nc.vector.tensor_copy(
                            absa[:, : ntf * H].rearrange("p (t h) -> p t h", h=H),
                            abv[:, :ntf, :H])
                        nc.vector.tensor_copy(
                            absd[:, : ntf * H].rearrange("p (t h) -> p t h", h=H),
                            abv[:, :ntf, H:])
                    for t in range(ntf, nt):
                        w = min(P, cw - t * P)
                        nc.vector.tensor_copy(
                            absa[:w, t * H : (t + 1) * H], abv[:w, t, :H])
                        nc.vector.tensor_copy(
                            absd[:w, t * H : (t + 1) * H], abv[:w, t, H:])
                    if ntf:
                        nc.sync.dma_start(
                            out=atv[:, w0 : w0 + ntf, :],
                            in_=absa[:, : ntf * H].rearrange(
                                "p (t h) -> p t h", h=H))
                        nc.sync.dma_start(
                            out=btv[:, w0 : w0 + ntf, :],
                            in_=absd[:, : ntf * H].rearrange(
                                "p (t h) -> p t h", h=H))
                    if nt > ntf:
                        w = cw - ntf * P
                        nc.sync.dma_start(out=atv[:w, w0 + ntf, :],
                                          in_=absa[:w, ntf * H : nt * H])
                        nc.sync.dma_start(out=btv[:w, w0 + ntf, :],
                                          in_=absd[:w, ntf * H : nt * H])

            # ---- phase 1: edges ----
            for b in range(NBUCK):
                atab_v = (atab_hi if b >= 2 else atab_lo)[:, :]
                btab_v = (btab_hi if (b % 2) else btab_lo)[:, :]
                for gi in range(GI_PER_B):
                    attrT = gap.tile([1, NI], BF16, tag="attrT")
                    nc.sync.dma_start(out=attrT[:],
                                      in_=attrT_t[b, :, gi * NI : (gi + 1) * NI])
                    gslab = ((b * GI_PER_B + gi) * 2) * (NI // 16)
                    iga = idxgA[:, gslab : gslab + NI // 16]
                    igb = idxgA[:, gslab + NI // 16 : gslab + 2 * (NI // 16)]
                    isc = idxscA[:, (b * GI_PER_B + gi) * (NI // 16) :
                                 (b * GI_PER_B + gi + 1) * (NI // 16)]
                    ga = gap.tile([P, TILES_PER_GI * H], BF16, tag="ga")
                    gb = gap.tile([P, TILES_PER_GI * H], BF16, tag="gb")
                    if "nogather" not in ablate:
                        nc.gpsimd.dma_gather(
                            out_ap=ga[:].rearrange("p (b h) -> p b h", h=H),
                            in_ap=atab_v, idxs_ap=iga, num_idxs=NI,
                            num_idxs_reg=NI, elem_size=H,
                            single_packet=False, queue_num=(2 * gi) % 4)
                        nc.gpsimd.dma_gather(
                            out_ap=gb[:].rearrange("p (b h) -> p b h", h=H),
                            in_ap=btab_v, idxs_ap=igb, num_idxs=NI,
                            num_idxs_reg=NI, elem_size=H,
                            single_packet=False, queue_num=(2 * gi + 1) % 4)
                    else:
                        nc.vector.memset(ga[:, :1], 0.0)
                        nc.vector.memset(gb[:, :1], 0.0)

                    src = srp.tile([P, TILES_PER_GI * 4], F32, tag="src")
                    if "nomlp" in ablate:
                        nc.vector.memset(src[:], 0.0)
                    for blk in range(0 if "nomlp" in ablate else TILES_PER_GI // 4):
                        tloc0 = blk * 4                    # sub-tile within gather
                        tglob0 = b * TB + gi * TILES_PER_GI + tloc0
                        pre = psp.tile([H, 512], F32, space="PSUM", tag="pre")
                        s4 = blp.tile([P, 4 * H], F32, tag="s4")
                        nc.vector.tensor_add(
                            s4[:],
                            ga[:, tloc0 * H : (tloc0 + 4) * H],
                            gb[:, tloc0 * H : (tloc0 + 4) * H])
                        for k in range(4):
                            nc.tensor.transpose(
                                out=pre[:, k * H : (k + 1) * H],
                                in_=s4[:, k * H : (k + 1) * H],
                                identity=ident[:])
                        e0 = tloc0 * P
                        nc.tensor.matmul(
                            pre[:, :], lhsT=w1c[:],
                            rhs=attrT[:, e0 : e0 + 512],
                            start=False, stop=True, skip_group_check=True)
                        x1 = blp.tile([H, 512], BF16, tag="x1")
                        nc.scalar.activation(
                            x1[:], pre[:],
                            mybir.ActivationFunctionType.Silu, bias=b1[:, :1])
                        pre2 = psp.tile([H, 512], F32, space="PSUM", tag="pre2")
                        nc.tensor.matmul(pre2[:], lhsT=w2T[:], rhs=x1[:],
                                         start=True, stop=True)
                        x2 = blp.tile([H, 512], BF16, tag="x2")
                        nc.scalar.activation(
                            x2[:], pre2[:],
                            mybir.ActivationFunctionType.Silu, bias=b2[:, :1])
                        phi4 = phps.tile([P, 4], F32, space="PSUM", tag="phi4")
                        for k in range(4):
                            nc.tensor.matmul(
                                phi4[:, k : k + 1],
                                lhsT=x2[:, k * H : (k + 1) * H],
                                rhs=w3Ts[:], start=True, stop=True)
                        nc.vector.tensor_mul(
                            src[:, tloc0 * 4 : (tloc0 + 4) * 4].rearrange(
                                "p (t c) -> p t c", c=4),
                            cdmR[:, tglob0 * 4 : (tglob0 + 4) * 4].rearrange(
                                "p (t c) -> p t c", c=4),
                            phi4[:].rearrange("p (t c) -> p t c", c=1
                                              ).broadcast_to([P, 4, 4]))
                    if "noscatter" not in ablate:
                        sset = surf[gi % NSETS]
                        own, oth = (sset[0], sset[1]) if b < 2 else (sset[2], sset[3])
                        nc.gpsimd.dma_scatter_add(
                            out_ap=own[:].rearrange("p (g c) -> p g c", c=4),
                            in_ap=src[:].rearrange("p (t c) -> p t c", c=4),
                            idxs_ap=isc[:],
                            num_idxs=NI,
                            num_idxs_reg=NI,
                            elem_size=4,
                            sbuf_tokens_per_rank=P,
                            parity_reg=0,
                            out_ap_other=oth[:].rearrange("p (g c) -> p g c", c=4),
                            queue_num=gi % 4)

            # ---- merge surface sets, dump raw surfaces ----
            for i in range(4):
                acc = surf[0][i]
                for s in range(1, NSETS):
                    nc.vector.tensor_add(acc[:], acc[:], surf[s][i][:])
                nc.sync.dma_start(out=aggs_t[:, i * G4 : (i + 1) * G4], in_=acc[:])
            loop_cm.__exit__(None, None, None)

    nc.finalize()
    return nc


def _tiled(m, w):
    """Node offset within a half -> flat row in the tiled table layout."""
    return (m % P) * w + m // P


def _prep_core(rows, cols, cdm, attr):
    """Reorder one core's edges into bucketed layout.

    Returns gather-idx [NBUCK, GI_PER_B, 2, 128, NI/16] i16,
    scatter-idx [NBUCK, GI_PER_B, 128, NI/16] i16,
    cdm [128, TTOT*4] bf16, attrT [NBUCK, 1, BCAP] bf16.
    """
    idxg = np.zeros((NBUCK, GI_PER_B, 2, P, NI // 16), np.int16)
    idxsc = np.zeros((NBUCK, GI_PER_B, P, NI // 16), np.int16)
    cdmR = np.zeros((TTOT, P, 4), ml_dtypes.bfloat16)
    attrT = np.zeros((NBUCK, 1, BCAP), ml_dtypes.bfloat16)
    bucket = (rows >= HALF).astype(np.int64) * 2 + (cols >= HALF).astype(np.int64)
    for b in range(NBUCK):
        sel = np.nonzero(bucket == b)[0]
        eb = len(sel)
        assert eb <= BCAP, f"bucket {b} has {eb} edges > cap {BCAP}"
        ra_f = np.zeros(BCAP, np.int16)         # gather idx into A half-table
        cb_f = np.zeros(BCAP, np.int16)         # gather idx into B half-table
        sc_f = np.zeros(BCAP, np.int16)         # scatter idx (0 = dump)
        cdm_f = np.zeros((BCAP, 4), np.float32)
        at_f = np.zeros(BCAP, np.float32)
        mr = rows[sel] - (HALF if b >= 2 else 0)
        mc = cols[sel] - (HALF if b % 2 else 0)
        ra_f[:eb] = _tiled(mr, WHI if b >= 2 else WLO).astype(np.int16)
        cb_f[:eb] = _tiled(mc, WHI if b % 2 else WLO).astype(np.int16)
        sc_f[:eb] = (mr + 256).astype(np.int16)
        cdm_f[:eb, :3] = cdm[sel]
        at_f[:eb] = attr[sel]
        # token i of each NI batch lives at [i%128, i//128] of the tile group
        bt0 = b * TB
        cdmR[bt0 : bt0 + TB] = cdm_f.reshape(TB, P, 4)
        attrT[b, 0, :] = at_f.astype(ml_dtypes.bfloat16)
        for gi in range(GI_PER_B):
            seg = slice(gi * NI, (gi + 1) * NI)
            idxg[b, gi, 0] = _wrap_idx(ra_f[seg])
            idxg[b, gi, 1] = _wrap_idx(cb_f[seg])
            idxsc[b, gi] = _wrap_idx(sc_f[seg])
    return (idxg, idxsc,
            np.ascontiguousarray(cdmR.transpose(1, 0, 2)).reshape(P, TTOT * 4),
            attrT)


def _decode_aggs(buf):
    """[128, 4*G4] raw surfaces -> [N, 3] f32 partial aggregate."""
    surfs = [buf[:, i * G4 : (i + 1) * G4].reshape(P, G, 4) for i in range(4)]
    agg = np.zeros((N, 3), np.float32)
    for base, hi, s0, s1 in ((0, NLO, surfs[0], surfs[1]),
                             (NLO, N, surfs[2], surfs[3])):
        n = np.arange(base, hi)
        idx = (n - base) + 256
        p = idx & 127
        slot = idx >> 7
        g = slot >> 1
        pair = np.stack([s0, s1])          # [2, 128, G, 4]
        agg[n] = pair[slot & 1, p, g, :3]
    return agg


def kernel(h, coord, edge_index, coord_diff, edge_attr, node_mask, edge_mask,
           W1, b1, W2, b2, W3):
    h = np.asarray(h, np.float32)
    coord = np.asarray(coord, np.float32)
    edge_index = np.asarray(edge_index)
    coord_diff = np.asarray(coord_diff, np.float32)
    edge_attr = np.asarray(edge_attr, np.float32)
    node_mask = np.asarray(node_mask, np.float32)
    edge_mask = np.asarray(edge_mask, np.float32)
    W1 = np.asarray(W1, np.float32)
    b1 = np.asarray(b1, np.float32)
    W2 = np.asarray(W2, np.float32)
    b2 = np.asarray(b2, np.float32)
    W3 = np.asarray(W3, np.float32)

    rows = edge_index[0].astype(np.int32)
    cols = edge_index[1].astype(np.int32)
    cdm = coord_diff * edge_mask  # fold edge mask

    bf = ml_dtypes.bfloat16
    hT = np.ascontiguousarray(h.T).astype(bf)
    w1abT = np.ascontiguousarray(
        np.concatenate([W1[:, :H].T, W1[:, H : 2 * H].T], axis=1)).astype(bf)
    w1c = np.ascontiguousarray(W1[:, 2 * H][None, :]).astype(bf)
    w2T = np.ascontiguousarray(W2.T).astype(bf)
    w3Ts = np.ascontiguousarray(W3.T / 100.0).astype(bf)
    b1c = np.ascontiguousarray(b1[:, None]).astype(np.float32)
    b2c = np.ascontiguousarray(b2[:, None]).astype(np.float32)

    if "nc" not in _nc_cache:
        _nc_cache["nc"] = _build_program()
    nc = _nc_cache["nc"]

    in_maps = []
    for c in range(NCORES):
        sl = slice(c * ECORE, (c + 1) * ECORE)
        idxg, idxsc, cdmR, attrT = _prep_core(
            rows[sl], cols[sl], cdm[sl], edge_attr[sl, 0])
        in_maps.append({
            "hT": hT, "w1abT": w1abT, "w1c": w1c, "w2T": w2T, "w3Ts": w3Ts,
            "b1c": b1c, "b2c": b2c,
            "idxg": idxg, "idxsc": idxsc, "cdm": cdmR, "attrT": attrT,
        })

    res = run_bass_kernel_spmd(nc, in_maps, list(range(NCORES))).results
    agg = np.zeros((N, 3), np.float32)
    for c in range(NCORES):
        agg += _decode_aggs(res[c]["aggs"])
    return (coord + agg) * node_mask


# revision 48
# speedup vs baseline: 3.4035x; 3.4035x over previous
"""EquivariantUpdate Bass kernel for 8 TRN2 NeuronCores.

Strategy (edge-sharded, per sharding hint):
- Host: shard E=800k edges 8 ways; per core, bucket edges by
  (row>=25000, col>=25000) so dma_gather's int16 indices work against
  half-table views. No sorting needed.
- Device: A = h @ W1a.T, B = h @ W1b.T tables in bf16; dma_gather
  A[row], B[col] over 4 SWDGE queues; per-edge MLP on PE/ACT/DVE;
  trans = (coord_diff*edge_mask) * phi; segment-sum via
  dma_scatter_add with SBUF destination (CCE add into on-chip
  accumulator surfaces). The row-half bucket split keeps scatter
  indices int16: rows<25000 go to surface pair A (idx=row+256),
  rows>=25000 to pair B (idx=row-25000+256); idx group 0 is a dump
  column for pad edges. Parity (bit 7 of idx) splits each pair into
  own/other surfaces per the HW decode. Two rotating surface sets
  break the WAW chain between consecutive scatters; merged on-chip
  at the end and DMA'd out as raw surfaces.
- Host: decode surfaces -> [N,3], sum the 8 per-core partials,
  out = (coord + agg) * node_mask.
  (1/NORM_FACTOR is folded into W3; edge_mask folded into coord_diff.)
"""
import numpy as np
import ml_dtypes

import concourse.bass as bass
import concourse.bacc as bacc
import concourse.mybir as mybir
import concourse.tile as tile
from concourse.bass_utils import run_bass_kernel_spmd
from concourse.masks import make_identity

P = 128
N = 50000
H = 128
E = 800000
NCORES = 8
ECORE = E // NCORES          # 100000
HALF = 24960                 # table split point (multiple of 128, int16-safe)
NLO = HALF                   # nodes in low half
NHI = N - HALF               # 25040 nodes in high half
WLO = NLO // P               # 195 windows (tiled table layout)
WHI = (NHI + P - 1) // P     # 196 windows
NHIP = WHI * P               # 25088 padded rows for the high half
NI = 2048                    # wrap-block granularity (host layout)
TILES_PER_GI = NI // P       # 16
BCAP = 26624                 # padded edges per bucket (13 * 2048)
GI_PER_B = BCAP // NI        # 13
TB = BCAP // P               # 208 tiles per bucket
# device-side batch sizes per bucket: 6x4096 + 1x2048 = 26624
GJ_SIZES = (4096,) * 6 + (2048,)
NBUCK = 4
TTOT = NBUCK * TB            # 832 tiles per core
G = 99                       # scatter groups per surface (idx<=25255 -> g<=98)
G4 = G * 4                   # f32 words per partition per surface
NSETS = 4                    # rotating surface sets (break WAW chains)

BF16 = mybir.dt.bfloat16
F32 = mybir.dt.float32
F8 = mybir.dt.float8e4
I16 = mybir.dt.int16
I32 = mybir.dt.int32

_nc_cache = {}


def _wrap_idx(idx_flat):
    """int16 indices -> wrapped [16, NI/16] replicated to [128, NI/16]."""
    w = idx_flat.reshape(NI // 16, 16).T.astype(np.int16)
    return np.tile(w, (8, 1))


def _build_program(loop_k=0, ablate=None):
    import os
    ablate = ablate or os.environ.get("KABLATE", "")
    import contextlib
    nc = bacc.Bacc(None, target_bir_lowering=False, num_swdge_queues=4)

    # ---- inputs (per-core values, same shapes everywhere) ----
    hT_t = nc.dram_tensor("hT", [H, N], BF16, kind="ExternalInput")
    w1abT_t = nc.dram_tensor("w1abT", [H, 2 * H], BF16, kind="ExternalInput")
    w1c_t = nc.dram_tensor("w1c", [1, H], BF16, kind="ExternalInput")
    w2T_t = nc.dram_tensor("w2T", [H, H], BF16, kind="ExternalInput")
    w3Ts_t = nc.dram_tensor("w3Ts", [H, 1], BF16, kind="ExternalInput")
    b1_t = nc.dram_tensor("b1c", [H, 1], F32, kind="ExternalInput")
    b2_t = nc.dram_tensor("b2c", [H, 1], F32, kind="ExternalInput")
    # gather indices: per bucket, [A-cols | B-cols] wrapped int16 slabs
    idxg_t = nc.dram_tensor("idxg", [NBUCK, 2, P, GI_PER_B * (NI // 16)], I16,
                            kind="ExternalInput")
    # scatter indices: per bucket wrapped token idx into surface pair
    idxsc_t = nc.dram_tensor("idxsc", [NBUCK, P, GI_PER_B * (NI // 16)], I16,
                             kind="ExternalInput")
    cdm_t = nc.dram_tensor("cdm", [P, TTOT * 4], BF16, kind="ExternalInput")
    attrT_t = nc.dram_tensor("attrT", [NBUCK, 1, BCAP], BF16, kind="ExternalInput")

    aggs_t = nc.dram_tensor("aggs", [P, 4 * G4], F32, kind="ExternalOutput")

    # node tables in tiled layout: node m of a half lives at flat row
    # (m % 128) * W + m // 128, so phase-0 writes are >=1KB per partition
    atab_lo = nc.dram_tensor("atab_lo", [NLO, H], BF16)
    atab_hi = nc.dram_tensor("atab_hi", [NHIP, H], BF16)
    btab_lo = nc.dram_tensor("btab_lo", [NLO, H], BF16)
    btab_hi = nc.dram_tensor("btab_hi", [NHIP, H], BF16)

    NT_N = (N + P - 1) // P  # 391 node tiles
    with tile.TileContext(nc) as tc:
        with (
            tc.tile_pool(name="static", bufs=1) as stp,
            tc.tile_pool(name="p0", bufs=4) as p0p,
            tc.tile_pool(name="p0ps", bufs=1, space="PSUM") as p0ps,
            tc.tile_pool(name="gat", bufs=2) as gap,
            tc.tile_pool(name="blk", bufs=3) as blp,
            tc.tile_pool(name="ps", bufs=2, space="PSUM") as psp,
            tc.tile_pool(name="phips", bufs=2, space="PSUM") as phps,
            tc.tile_pool(name="sc", bufs=8) as scp,
            tc.tile_pool(name="src", bufs=3) as srp,
        ):
            # ---- statics ----
            ident = stp.tile([P, P], F32)
            make_identity(nc, ident[:])
            w1abT = stp.tile([H, 2 * H], BF16)
            nc.sync.dma_start(out=w1abT[:], in_=w1abT_t[:, :])
            w1c = stp.tile([1, H], BF16)
            nc.sync.dma_start(out=w1c[:], in_=w1c_t[:, :])
            w2T = stp.tile([H, H], BF16)
            nc.sync.dma_start(out=w2T[:], in_=w2T_t[:, :])
            w3Ts = stp.tile([H, 1], BF16)
            nc.sync.dma_start(out=w3Ts[:], in_=w3Ts_t[:, :])
            b1 = stp.tile([H, 1], F32)
            nc.sync.dma_start(out=b1[:], in_=b1_t[:, :])
            b2 = stp.tile([H, 1], F32)
            nc.sync.dma_start(out=b2[:], in_=b2_t[:, :])
            cdmR = stp.tile([P, TTOT * 4], BF16)
            nc.sync.dma_start(out=cdmR[:], in_=cdm_t[:, :])
            # index slabs: one DMA each instead of 3 per gather batch
            idxgA = stp.tile([P, NBUCK * 2 * GI_PER_B * (NI // 16)], I16)
            nc.sync.dma_start(
                out=idxgA[:].rearrange("p (n f) -> p n f", n=NBUCK * 2),
                in_=idxg_t[:, :, :, :].rearrange("n t p f -> p (n t) f"))
            idxscA = stp.tile([P, NBUCK * GI_PER_B * (NI // 16)], I16)
            nc.sync.dma_start(
                out=idxscA[:].rearrange("p (n f) -> p n f", n=NBUCK),
                in_=idxsc_t[:, :, :].rearrange("n p f -> p n f"))

            # ---- accumulator surfaces: [set][pair a/b][parity 0/1] ----
            surf = [[stp.tile([P, G4], F32, tag=f"surf{s}{i}", name=f"surf{s}{i}")
                     for i in range(4)]
                    for s in range(NSETS)]

            loop_cm = tc.For_i(0, loop_k, 1) if loop_k else contextlib.nullcontext()
            loop_cm.__enter__()
            for s in range(NSETS):
                for i in range(4):
                    nc.vector.memset(surf[s][i][:], 0.0)

            # ---- phase 0: A/B tables (512-node chunks, tiled-layout writes) ----
            NCHUNK = 512
            halves = ((0, NLO, atab_lo, btab_lo), (NLO, NHI, atab_hi, btab_hi))
            if "nophase0" in ablate:
                halves = ()
            for base, cnt, at_t, bt_t in halves:
                atv = at_t[:, :].rearrange("(p w) h -> p w h", p=P)
                btv = bt_t[:, :].rearrange("(p w) h -> p w h", p=P)
                for c0 in range(0, cnt, NCHUNK):
                    cw = min(NCHUNK, cnt - c0)
                    nt = (cw + P - 1) // P
                    ntf = cw // P
                    w0 = c0 // P
                    ht = p0p.tile([H, NCHUNK], BF16, tag="ht")
                    nc.sync.dma_start(out=ht[:, :cw],
                                      in_=hT_t[:, base + c0 : base + c0 + cw])
                    absa = p0p.tile([P, (NCHUNK // P) * H], BF16, tag="absa")
                    absd = p0p.tile([P, (NCHUNK // P) * H], BF16, tag="absd")
                    ab = p0ps.tile([P, (NCHUNK // P) * 2 * H], F32, space="PSUM",
                                   tag="abps")
                    for t in range(nt):
                        w = min(P, cw - t * P)
                        nc.tensor.matmul(ab[:w, t * 2 * H : (t + 1) * 2 * H],
                                         lhsT=ht[:, t * P : t * P + w],
                                         rhs=w1abT[:], start=True, stop=True)
                    abv = ab[:].rearrange("p (t f) -> p t f", f=2 * H)
                    if ntf:
                        nc.vector.tensor_copy(
                            absa[:, : ntf * H].rearrange("p (t h) -> p t h", h=H),
                            abv[:, :ntf, :H])
                        nc.vector.tensor_copy(
                            absd[:, : ntf * H].rearrange("p (t h) -> p t h", h=H),
                            abv[:, :ntf, H:])
                    for t in range(ntf, nt):
                        w = min(P, cw - t * P)
                        nc.vector.tensor_copy(
                            absa[:w, t * H : (t + 1) * H], abv[:w, t, :H])
                        nc.vector.tensor_copy(
                            absd[:w, t * H : (t + 1) * H], abv[:w, t, H:])
                    if ntf:
                        nc.sync.dma_start(
                            out=atv[:, w0 : w0 + ntf, :],
                            in_=absa[:, : ntf * H].rearrange(
                                "p (t h) -> p t h", h=H))
                        nc.sync.dma_start(
                            out=btv[:, w0 : w0 + ntf, :],
                            in_=absd[:, : ntf * H].rearrange(
                                "p (t h) -> p t h", h=H))
                    if nt > ntf:
                        w = cw - ntf * P
                        nc.sync.dma_start(out=atv[:w, w0 + ntf, :],
                                          in_=absa[:w, ntf * H : nt * H])
                        nc.sync.dma_start(out=btv[:w, w0 + ntf, :],
                                          in_=absd[:w, ntf * H : nt * H])

            # ---- phase 1: edges ----
            MAXT = max(GJ_SIZES) // P        # 32 tiles per device batch
            sp = "sp1" in ablate
            gctr = 0
            for b in range(NBUCK):
                atab_v = (atab_hi if b >= 2 else atab_lo)[:, :]
                btab_v = (btab_hi if (b % 2) else btab_lo)[:, :]
                e0b = 0
                for gj, sz in enumerate(GJ_SIZES):
                    nt = sz // P                    # tiles in this batch
                    c0 = e0b // 16                  # idx-slab column offset
                    bb = b * GI_PER_B * (NI // 16)  # bucket base in slab cols
                    iga = idxgA[:, 2 * bb + c0 : 2 * bb + c0 + sz // 16]
                    igb = idxgA[:, 2 * bb + GI_PER_B * (NI // 16) + c0 :
                                2 * bb + GI_PER_B * (NI // 16) + c0 + sz // 16]
                    isc = idxscA[:, bb + c0 : bb + c0 + sz // 16]
                    attrT = gap.tile([1, max(GJ_SIZES)], BF16, tag="attrT")
                    nc.sync.dma_start(out=attrT[:, :sz],
                                      in_=attrT_t[b, :, e0b : e0b + sz])
                    ga = gap.tile([P, MAXT * H], BF16, tag="ga")
                    gb = gap.tile([P, MAXT * H], BF16, tag="gb")
                    if "nogather" not in ablate:
                        nc.gpsimd.dma_gather(
                            out_ap=ga[:, : nt * H].rearrange(
                                "p (b h) -> p b h", h=H),
                            in_ap=atab_v, idxs_ap=iga, num_idxs=sz,
                            num_idxs_reg=sz, elem_size=H,
                            single_packet=sp, queue_num=(2 * gctr) % 4)
                        nc.gpsimd.dma_gather(
                            out_ap=gb[:, : nt * H].rearrange(
                                "p (b h) -> p b h", h=H),
                            in_ap=btab_v, idxs_ap=igb, num_idxs=sz,
                            num_idxs_reg=sz, elem_size=H,
                            single_packet=sp, queue_num=(2 * gctr + 1) % 4)
                    else:
                        nc.vector.memset(ga[:, :1], 0.0)
                        nc.vector.memset(gb[:, :1], 0.0)

                    src = srp.tile([P, MAXT * 4], F32, tag="src")
                    phiB = phps.tile([P, MAXT], F32, space="PSUM", tag="phiB")
                    if "nomlp" in ablate:
                        nc.vector.memset(src[:], 0.0)
                    for blk in range(0 if "nomlp" in ablate else nt // 4):
                        tloc0 = blk * 4
                        pre = psp.tile([H, 512], F32, space="PSUM", tag="pre")
                        s4 = blp.tile([P, 4 * H], F32, tag="s4")
                        nc.vector.tensor_add(
                            s4[:],
                            ga[:, tloc0 * H : (tloc0 + 4) * H],
                            gb[:, tloc0 * H : (tloc0 + 4) * H])
                        for k in range(4):
                            nc.tensor.transpose(
                                out=pre[:, k * H : (k + 1) * H],
                                in_=s4[:, k * H : (k + 1) * H],
                                identity=ident[:])
                        nc.tensor.matmul(
                            pre[:, :], lhsT=w1c[:],
                            rhs=attrT[:, tloc0 * P : tloc0 * P + 512],
                            start=False, stop=True, skip_group_check=True)
                        x1 = blp.tile([H, 512], BF16, tag="x1")
                        nc.scalar.activation(
                            x1[:], pre[:],
                            mybir.ActivationFunctionType.Silu, bias=b1[:, :1])
                        pre2 = psp.tile([H, 512], F32, space="PSUM", tag="pre2")
                        nc.tensor.matmul(pre2[:], lhsT=w2T[:], rhs=x1[:],
                                         start=True, stop=True)
                        x2 = blp.tile([H, 512], BF16, tag="x2")
                        nc.scalar.activation(
                            x2[:], pre2[:],
                            mybir.ActivationFunctionType.Silu, bias=b2[:, :1])
                        for k in range(4):
                            nc.tensor.matmul(
                                phiB[:, tloc0 + k : tloc0 + k + 1],
                                lhsT=x2[:, k * H : (k + 1) * H],
                                rhs=w3Ts[:], start=True, stop=True)
                    if "nomlp" not in ablate:
                        tg0 = b * TB + e0b // P
                        nc.vector.tensor_mul(
                            src[:, : nt * 4].rearrange("p (t c) -> p t c", c=4),
                            cdmR[:, tg0 * 4 : (tg0 + nt) * 4].rearrange(
                                "p (t c) -> p t c", c=4),
                            phiB[:, :nt].rearrange("p (t c) -> p t c", c=1
                                                   ).broadcast_to([P, nt, 4]))
                    if "noscatter" not in ablate:
                        sset = surf[gctr % NSETS]
                        own, oth = (sset[0], sset[1]) if b < 2 else (sset[2], sset[3])
                        nc.gpsimd.dma_scatter_add(
                            out_ap=own[:].rearrange("p (g c) -> p g c", c=4),
                            in_ap=src[:, : nt * 4].rearrange(
                                "p (t c) -> p t c", c=4),
                            idxs_ap=isc[:],
                            num_idxs=sz,
                            num_idxs_reg=sz,
                            elem_size=4,
                            sbuf_tokens_per_rank=P,
                            parity_reg=0,
                            out_ap_other=oth[:].rearrange("p (g c) -> p g c", c=4),
                            queue_num=gctr % 4)
                    e0b += sz
                    gctr += 1

            # ---- merge surface sets, dump raw surfaces ----
            for i in range(4):
                acc = surf[0][i]
                for s in range(1, NSETS):
                    nc.vector.tensor_add(acc[:], acc[:], surf[s][i][:])
                nc.sync.dma_start(out=aggs_t[:, i * G4 : (i + 1) * G4], in_=acc[:])
            loop_cm.__exit__(None, None, None)

    nc.finalize()
    return nc


def _tiled(m, w):
    """Node offset within a half -> flat row in the tiled table layout."""
    return (m % P) * w + m // P


def _prep_core(rows, cols, cdm, attr):
    """Reorder one core's edges into bucketed layout.

    Returns gather-idx [NBUCK, GI_PER_B, 2, 128, NI/16] i16,
    scatter-idx [NBUCK, GI_PER_B, 128, NI/16] i16,
    cdm [128, TTOT*4] bf16, attrT [NBUCK, 1, BCAP] bf16.
    """
    idxg = np.zeros((NBUCK, 2, P, GI_PER_B * (NI // 16)), np.int16)
    idxsc = np.zeros((NBUCK, P, GI_PER_B * (NI // 16)), np.int16)
    cdmR = np.zeros((TTOT, P, 4), ml_dtypes.bfloat16)
    attrT = np.zeros((NBUCK, 1, BCAP), ml_dtypes.bfloat16)
    bucket = (rows >= HALF).astype(np.int64) * 2 + (cols >= HALF).astype(np.int64)
    for b in range(NBUCK):
        sel = np.nonzero(bucket == b)[0]
        eb = len(sel)
        assert eb <= BCAP, f"bucket {b} has {eb} edges > cap {BCAP}"
        # order edges by (row block, col) for DRAM locality in both gathers
        sel = sel[np.lexsort((cols[sel], rows[sel] >> 9))]
        ra_f = np.zeros(BCAP, np.int16)         # gather idx into A half-table
        cb_f = np.zeros(BCAP, np.int16)         # gather idx into B half-table
        sc_f = np.zeros(BCAP, np.int16)         # scatter idx (0 = dump)
        cdm_f = np.zeros((BCAP, 4), np.float32)
        at_f = np.zeros(BCAP, np.float32)
        mr = rows[sel] - (HALF if b >= 2 else 0)
        mc = cols[sel] - (HALF if b % 2 else 0)
        ra_f[:eb] = _tiled(mr, WHI if b >= 2 else WLO).astype(np.int16)
        cb_f[:eb] = _tiled(mc, WHI if b % 2 else WLO).astype(np.int16)
        sc_f[:eb] = (mr + 256).astype(np.int16)
        cdm_f[:eb, :3] = cdm[sel]
        at_f[:eb] = attr[sel]
        # token i of each NI batch lives at [i%128, i//128] of the tile group
        bt0 = b * TB
        cdmR[bt0 : bt0 + TB] = cdm_f.reshape(TB, P, 4)
        attrT[b, 0, :] = at_f.astype(ml_dtypes.bfloat16)
        for gi in range(GI_PER_B):
            seg = slice(gi * NI, (gi + 1) * NI)
            csl = slice(gi * (NI // 16), (gi + 1) * (NI // 16))
            idxg[b, 0, :, csl] = _wrap_idx(ra_f[seg])
            idxg[b, 1, :, csl] = _wrap_idx(cb_f[seg])
            idxsc[b, :, csl] = _wrap_idx(sc_f[seg])
    return (idxg, idxsc,
            np.ascontiguousarray(cdmR.transpose(1, 0, 2)).reshape(P, TTOT * 4),
            attrT)


def _decode_aggs(buf):
    """[128, 4*G4] raw surfaces -> [N, 3] f32 partial aggregate."""
    surfs = [buf[:, i * G4 : (i + 1) * G4].reshape(P, G, 4) for i in range(4)]
    agg = np.zeros((N, 3), np.float32)
    for base, hi, s0, s1 in ((0, NLO, surfs[0], surfs[1]),
                             (NLO, N, surfs[2], surfs[3])):
        n = np.arange(base, hi)
        idx = (n - base) + 256
        p = idx & 127
        slot = idx >> 7
        g = slot >> 1
        pair = np.stack([s0, s1])          # [2, 128, G, 4]
        agg[n] = pair[slot & 1, p, g, :3]
    return agg


def kernel(h, coord, edge_index, coord_diff, edge_attr, node_mask, edge_mask,
           W1, b1, W2, b2, W3):
    h = np.asarray(h, np.float32)
    coord = np.asarray(coord, np.float32)
    edge_index = np.asarray(edge_index)
    coord_diff = np.asarray(coord_diff, np.float32)
    edge_attr = np.asarray(edge_attr, np.float32)
    node_mask = np.asarray(node_mask, np.float32)
    edge_mask = np.asarray(edge_mask, np.float32)
    W1 = np.asarray(W1, np.float32)
    b1 = np.asarray(b1, np.float32)
    W2 = np.asarray(W2, np.float32)
    b2 = np.asarray(b2, np.float32)
    W3 = np.asarray(W3, np.float32)

    rows = edge_index[0].astype(np.int32)
    cols = edge_index[1].astype(np.int32)
    cdm = coord_diff * edge_mask  # fold edge mask

    bf = ml_dtypes.bfloat16
    hT = np.ascontiguousarray(h.T).astype(bf)
    w1abT = np.ascontiguousarray(
        np.concatenate([W1[:, :H].T, W1[:, H : 2 * H].T], axis=1)).astype(bf)
    w1c = np.ascontiguousarray(W1[:, 2 * H][None, :]).astype(bf)
    w2T = np.ascontiguousarray(W2.T).astype(bf)
    w3Ts = np.ascontiguousarray(W3.T / 100.0).astype(bf)
    b1c = np.ascontiguousarray(b1[:, None]).astype(np.float32)
    b2c = np.ascontiguousarray(b2[:, None]).astype(np.float32)

    if "nc" not in _nc_cache:
        _nc_cache["nc"] = _build_program()
    nc = _nc_cache["nc"]

    in_maps = []
    for c in range(NCORES):
        sl = slice(c * ECORE, (c + 1) * ECORE)
        idxg, idxsc, cdmR, attrT = _prep_core(
            rows[sl], cols[sl], cdm[sl], edge_attr[sl, 0])
        in_maps.append({
            "hT": hT, "w1abT": w1abT, "w1c": w1c, "w2T": w2T, "w3Ts": w3Ts,
            "b1c": b1c, "b2c": b2c,
            "idxg": idxg, "idxsc": idxsc, "cdm": cdmR, "attrT": attrT,
        })

    res = run_bass_kernel_spmd(nc, in_maps, list(range(NCORES))).results
    agg = np.zeros((N, 3), np.float32)
    for c in range(NCORES):
        agg += _decode_aggs(res[c]["aggs"])
    return (coord + agg) * node_mask


# revision 54
# speedup vs baseline: 6.1204x; 1.7983x over previous
"""EquivariantUpdate Bass kernel for 8 TRN2 NeuronCores.

Strategy (edge-sharded, per sharding hint):
- Host: shard E=800k edges 8 ways; per core, bucket edges by
  (row>=25000, col>=25000) so dma_gather's int16 indices work against
  half-table views. No sorting needed.
- Device: A = h @ W1a.T, B = h @ W1b.T tables in bf16; dma_gather
  A[row], B[col] over 4 SWDGE queues; per-edge MLP on PE/ACT/DVE;
  trans = (coord_diff*edge_mask) * phi; segment-sum via
  dma_scatter_add with SBUF destination (CCE add into on-chip
  accumulator surfaces). The row-half bucket split keeps scatter
  indices int16: rows<25000 go to surface pair A (idx=row+256),
  rows>=25000 to pair B (idx=row-25000+256); idx group 0 is a dump
  column for pad edges. Parity (bit 7 of idx) splits each pair into
  own/other surfaces per the HW decode. Two rotating surface sets
  break the WAW chain between consecutive scatters; merged on-chip
  at the end and DMA'd out as raw surfaces.
- Host: decode surfaces -> [N,3], sum the 8 per-core partials,
  out = (coord + agg) * node_mask.
  (1/NORM_FACTOR is folded into W3; edge_mask folded into coord_diff.)
"""
import numpy as np
import ml_dtypes

import concourse.bass as bass
import concourse.bacc as bacc
import concourse.mybir as mybir
import concourse.tile as tile
from concourse.bass_utils import run_bass_kernel_spmd
from concourse.masks import make_identity

P = 128
N = 50000
H = 128
E = 800000
NCORES = 8
ECORE = E // NCORES          # 100000
HALF = 24960                 # table split point (multiple of 128, int16-safe)
NLO = HALF                   # nodes in low half
NHI = N - HALF               # 25040 nodes in high half
WLO = NLO // P               # 195 windows (tiled table layout)
WHI = (NHI + P - 1) // P     # 196 windows
NHIP = WHI * P               # 25088 padded rows for the high half
NI = 2048                    # wrap-block granularity (host layout)
TILES_PER_GI = NI // P       # 16
BCAP = 26624                 # padded edges per bucket (13 * 2048)
GI_PER_B = BCAP // NI        # 13
TB = BCAP // P               # 208 tiles per bucket
# device-side batch sizes per bucket: 6x4096 + 1x2048 = 26624
GJ_SIZES = (4096,) * 6 + (2048,)
NBUCK = 4
TTOT = NBUCK * TB            # 832 tiles per core
G = 99                       # scatter groups per surface (idx<=25255 -> g<=98)
G4 = G * 4                   # f32 words per partition per surface
NSETS = 4                    # rotating surface sets (break WAW chains)

BF16 = mybir.dt.bfloat16
F32 = mybir.dt.float32
F8 = mybir.dt.float8e4
I16 = mybir.dt.int16
I32 = mybir.dt.int32

_nc_cache = {}


def _wrap_idx(idx_flat):
    """int16 indices -> wrapped [16, NI/16] replicated to [128, NI/16]."""
    w = idx_flat.reshape(NI // 16, 16).T.astype(np.int16)
    return np.tile(w, (8, 1))


def _build_program(loop_k=0, ablate=None):
    import os
    ablate = ablate or os.environ.get("KABLATE", "")
    import contextlib
    nc = bacc.Bacc(None, target_bir_lowering=False, num_swdge_queues=4)

    # ---- inputs (per-core values, same shapes everywhere) ----
    hT_t = nc.dram_tensor("hT", [H, N], BF16, kind="ExternalInput")
    w1abT_t = nc.dram_tensor("w1abT", [H, 2 * H], BF16, kind="ExternalInput")
    w1c_t = nc.dram_tensor("w1c", [1, H], BF16, kind="ExternalInput")
    w2T_t = nc.dram_tensor("w2T", [H, H], BF16, kind="ExternalInput")
    w3Ts_t = nc.dram_tensor("w3Ts", [H, 1], BF16, kind="ExternalInput")
    b1_t = nc.dram_tensor("b1c", [H, 1], F32, kind="ExternalInput")
    b2_t = nc.dram_tensor("b2c", [H, 1], F32, kind="ExternalInput")
    # gather indices: per bucket, [A-cols | B-cols] wrapped int16 slabs
    idxg_t = nc.dram_tensor("idxg", [NBUCK, 2, P, GI_PER_B * (NI // 16)], I16,
                            kind="ExternalInput")
    # scatter indices: per bucket wrapped token idx into surface pair
    idxsc_t = nc.dram_tensor("idxsc", [NBUCK, P, GI_PER_B * (NI // 16)], I16,
                             kind="ExternalInput")
    cdm_t = nc.dram_tensor("cdm", [P, TTOT * 4], BF16, kind="ExternalInput")
    attrT_t = nc.dram_tensor("attrT", [NBUCK, 1, BCAP], BF16, kind="ExternalInput")

    aggs_t = nc.dram_tensor("aggs", [P, 4 * G4], F32, kind="ExternalOutput")

    # node tables in tiled layout: node m of a half lives at flat row
    # (m % 128) * W + m // 128, so phase-0 writes are >=1KB per partition
    atab_lo = nc.dram_tensor("atab_lo", [NLO, H], BF16)
    atab_hi = nc.dram_tensor("atab_hi", [NHIP, H], BF16)
    btab_lo = nc.dram_tensor("btab_lo", [NLO, H], BF16)
    btab_hi = nc.dram_tensor("btab_hi", [NHIP, H], BF16)

    NT_N = (N + P - 1) // P  # 391 node tiles
    deep = "nodeep" not in (ablate or "")
    with tile.TileContext(nc) as tc:
        with (
            tc.tile_pool(name="static", bufs=1) as stp,
            tc.tile_pool(name="p0", bufs=4) as p0p,
            tc.tile_pool(name="p0ps", bufs=1, space="PSUM") as p0ps,
            tc.tile_pool(name="gat", bufs=3 if deep else 2) as gap,
            tc.tile_pool(name="blk", bufs=6 if deep else 3) as blp,
            tc.tile_pool(name="ps", bufs=2, space="PSUM") as psp,
            tc.tile_pool(name="phips", bufs=2, space="PSUM") as phps,
            tc.tile_pool(name="sc", bufs=8) as scp,
            tc.tile_pool(name="src", bufs=4 if deep else 3) as srp,
        ):
            # ---- statics ----
            ident = stp.tile([P, P], F32)
            make_identity(nc, ident[:])
            identb = stp.tile([P, P], BF16)
            make_identity(nc, identb[:])
            w1abT = stp.tile([H, 2 * H], BF16)
            nc.sync.dma_start(out=w1abT[:], in_=w1abT_t[:, :])
            w1c = stp.tile([1, H], BF16)
            nc.sync.dma_start(out=w1c[:], in_=w1c_t[:, :])
            w2T = stp.tile([H, H], BF16)
            nc.sync.dma_start(out=w2T[:], in_=w2T_t[:, :])
            w3Ts = stp.tile([H, 1], BF16)
            nc.sync.dma_start(out=w3Ts[:], in_=w3Ts_t[:, :])
            b1 = stp.tile([H, 1], F32)
            nc.sync.dma_start(out=b1[:], in_=b1_t[:, :])
            b2 = stp.tile([H, 1], F32)
            nc.sync.dma_start(out=b2[:], in_=b2_t[:, :])
            cdmR = stp.tile([P, TTOT * 4], BF16)
            nc.sync.dma_start(out=cdmR[:], in_=cdm_t[:, :])
            # index slabs: one DMA each instead of 3 per gather batch
            idxgA = stp.tile([P, NBUCK * 2 * GI_PER_B * (NI // 16)], I16)
            nc.sync.dma_start(
                out=idxgA[:].rearrange("p (n f) -> p n f", n=NBUCK * 2),
                in_=idxg_t[:, :, :, :].rearrange("n t p f -> p (n t) f"))
            idxscA = stp.tile([P, NBUCK * GI_PER_B * (NI // 16)], I16)
            nc.sync.dma_start(
                out=idxscA[:].rearrange("p (n f) -> p n f", n=NBUCK),
                in_=idxsc_t[:, :, :].rearrange("n p f -> p n f"))

            # ---- accumulator surfaces: [set][pair a/b][parity 0/1] ----
            surf = [[stp.tile([P, G4], F32, tag=f"surf{s}{i}", name=f"surf{s}{i}")
                     for i in range(4)]
                    for s in range(NSETS)]

            loop_cm = tc.For_i(0, loop_k, 1) if loop_k else contextlib.nullcontext()
            loop_cm.__enter__()
            for s in range(NSETS):
                for i in range(4):
                    nc.vector.memset(surf[s][i][:], 0.0)

            # ---- phase 0: A/B tables (512-node chunks, tiled-layout writes) ----
            NCHUNK = 512
            halves = ((0, NLO, atab_lo, btab_lo), (NLO, NHI, atab_hi, btab_hi))
            if "nophase0" in ablate:
                halves = ()
            for base, cnt, at_t, bt_t in halves:
                atv = at_t[:, :].rearrange("(p w) h -> p w h", p=P)
                btv = bt_t[:, :].rearrange("(p w) h -> p w h", p=P)
                for c0 in range(0, cnt, NCHUNK):
                    cw = min(NCHUNK, cnt - c0)
                    nt = (cw + P - 1) // P
                    ntf = cw // P
                    w0 = c0 // P
                    ht = p0p.tile([H, NCHUNK], BF16, tag="ht")
                    nc.sync.dma_start(out=ht[:, :cw],
                                      in_=hT_t[:, base + c0 : base + c0 + cw])
                    absa = p0p.tile([P, (NCHUNK // P) * H], BF16, tag="absa")
                    absd = p0p.tile([P, (NCHUNK // P) * H], BF16, tag="absd")
                    ab = p0ps.tile([P, (NCHUNK // P) * 2 * H], F32, space="PSUM",
                                   tag="abps")
                    for t in range(nt):
                        w = min(P, cw - t * P)
                        nc.tensor.matmul(ab[:w, t * 2 * H : (t + 1) * 2 * H],
                                         lhsT=ht[:, t * P : t * P + w],
                                         rhs=w1abT[:], start=True, stop=True)
                    abv = ab[:].rearrange("p (t f) -> p t f", f=2 * H)
                    if ntf:
                        nc.vector.tensor_copy(
                            absa[:, : ntf * H].rearrange("p (t h) -> p t h", h=H),
                            abv[:, :ntf, :H])
                        nc.vector.tensor_copy(
                            absd[:, : ntf * H].rearrange("p (t h) -> p t h", h=H),
                            abv[:, :ntf, H:])
                    for t in range(ntf, nt):
                        w = min(P, cw - t * P)
                        nc.vector.tensor_copy(
                            absa[:w, t * H : (t + 1) * H], abv[:w, t, :H])
                        nc.vector.tensor_copy(
                            absd[:w, t * H : (t + 1) * H], abv[:w, t, H:])
                    if ntf:
                        nc.sync.dma_start(
                            out=atv[:, w0 : w0 + ntf, :],
                            in_=absa[:, : ntf * H].rearrange(
                                "p (t h) -> p t h", h=H))
                        nc.sync.dma_start(
                            out=btv[:, w0 : w0 + ntf, :],
                            in_=absd[:, : ntf * H].rearrange(
                                "p (t h) -> p t h", h=H))
                    if nt > ntf:
                        w = cw - ntf * P
                        nc.sync.dma_start(out=atv[:w, w0 + ntf, :],
                                          in_=absa[:w, ntf * H : nt * H])
                        nc.sync.dma_start(out=btv[:w, w0 + ntf, :],
                                          in_=absd[:w, ntf * H : nt * H])

            # ---- phase 1: edges ----
            MAXT = max(GJ_SIZES) // P        # 32 tiles per device batch
            sp = "sp1" in ablate
            gctr = 0
            for b in range(NBUCK):
                atab_v = (atab_hi if b >= 2 else atab_lo)[:, :]
                btab_v = (btab_hi if (b % 2) else btab_lo)[:, :]
                e0b = 0
                for gj, sz in enumerate(GJ_SIZES):
                    nt = sz // P                    # tiles in this batch
                    c0 = e0b // 16                  # idx-slab column offset
                    bb = b * GI_PER_B * (NI // 16)  # bucket base in slab cols
                    iga = idxgA[:, 2 * bb + c0 : 2 * bb + c0 + sz // 16]
                    igb = idxgA[:, 2 * bb + GI_PER_B * (NI // 16) + c0 :
                                2 * bb + GI_PER_B * (NI // 16) + c0 + sz // 16]
                    isc = idxscA[:, bb + c0 : bb + c0 + sz // 16]
                    attrT = gap.tile([1, max(GJ_SIZES)], BF16, tag="attrT")
                    nc.sync.dma_start(out=attrT[:, :sz],
                                      in_=attrT_t[b, :, e0b : e0b + sz])
                    tg = "notg" not in ablate
                    ga = gap.tile([P, MAXT * H], BF16, tag="ga")
                    gb = gap.tile([P, MAXT * H], BF16, tag="gb")
                    if "nogather" not in ablate:
                        if tg:
                            ga_o = ga[:, :sz].rearrange("p (o s) -> p o s", o=1)
                            gb_o = gb[:, :sz].rearrange("p (o s) -> p o s", o=1)
                        else:
                            ga_o = ga[:, : nt * H].rearrange("p (b h) -> p b h", h=H)
                            gb_o = gb[:, : nt * H].rearrange("p (b h) -> p b h", h=H)
                        nc.gpsimd.dma_gather(
                            out_ap=ga_o,
                            in_ap=atab_v, idxs_ap=iga, num_idxs=sz,
                            num_idxs_reg=sz, elem_size=H, transpose=tg,
                            single_packet=sp, queue_num=(2 * gctr) % 4)
                        nc.gpsimd.dma_gather(
                            out_ap=gb_o,
                            in_ap=btab_v, idxs_ap=igb, num_idxs=sz,
                            num_idxs_reg=sz, elem_size=H, transpose=tg,
                            single_packet=sp, queue_num=(2 * gctr + 1) % 4)
                    else:
                        nc.vector.memset(ga[:, :1], 0.0)
                        nc.vector.memset(gb[:, :1], 0.0)

                    src = srp.tile([P, MAXT * 4], F32, tag="src")
                    phiB = phps.tile([P, MAXT], F32, space="PSUM", tag="phiB")
                    if "nomlp" in ablate:
                        nc.vector.memset(src[:], 0.0)
                    for blk in range(0 if "nomlp" in ablate else nt // 4):
                        tloc0 = blk * 4
                        pre = psp.tile([H, 512], F32, space="PSUM", tag="pre")
                        if tg:
                            s4 = blp.tile([P, 4 * H], BF16, tag="s4")
                            nc.vector.tensor_add(
                                s4[:],
                                ga[:, tloc0 * P : (tloc0 + 4) * P],
                                gb[:, tloc0 * P : (tloc0 + 4) * P])
                            nc.tensor.matmul(
                                pre[:, :], lhsT=w1c[:],
                                rhs=attrT[:, tloc0 * P : tloc0 * P + 512],
                                start=True, stop=False, skip_group_check=True)
                            nc.tensor.matmul(
                                pre[:, :], lhsT=identb[:], rhs=s4[:],
                                start=False, stop=True, skip_group_check=True)
                        else:
                            s4 = blp.tile([P, 4 * H], F32, tag="s4")
                            nc.vector.tensor_add(
                                s4[:],
                                ga[:, tloc0 * H : (tloc0 + 4) * H],
                                gb[:, tloc0 * H : (tloc0 + 4) * H])
                            for k in range(4):
                                nc.tensor.transpose(
                                    out=pre[:, k * H : (k + 1) * H],
                                    in_=s4[:, k * H : (k + 1) * H],
                                    identity=ident[:])
                            nc.tensor.matmul(
                                pre[:, :], lhsT=w1c[:],
                                rhs=attrT[:, tloc0 * P : tloc0 * P + 512],
                                start=False, stop=True, skip_group_check=True)
                        x1 = blp.tile([H, 512], BF16, tag="x1")
                        nc.scalar.activation(
                            x1[:], pre[:],
                            mybir.ActivationFunctionType.Silu, bias=b1[:, :1])
                        pre2 = psp.tile([H, 512], F32, space="PSUM", tag="pre2")
                        nc.tensor.matmul(pre2[:], lhsT=w2T[:], rhs=x1[:],
                                         start=True, stop=True)
                        x2 = blp.tile([H, 512], BF16, tag="x2")
                        nc.scalar.activation(
                            x2[:], pre2[:],
                            mybir.ActivationFunctionType.Silu, bias=b2[:, :1])
                        for k in range(4):
                            nc.tensor.matmul(
                                phiB[:, tloc0 + k : tloc0 + k + 1],
                                lhsT=x2[:, k * H : (k + 1) * H],
                                rhs=w3Ts[:], start=True, stop=True)
                    if "nomlp" not in ablate:
                        tg0 = b * TB + e0b // P
                        nc.vector.tensor_mul(
                            src[:, : nt * 4].rearrange("p (t c) -> p t c", c=4),
                            cdmR[:, tg0 * 4 : (tg0 + nt) * 4].rearrange(
                                "p (t c) -> p t c", c=4),
                            phiB[:, :nt].rearrange("p (t c) -> p t c", c=1
                                                   ).broadcast_to([P, nt, 4]))
                    if "noscatter" not in ablate:
                        sset = surf[gctr % NSETS]
                        own, oth = (sset[0], sset[1]) if b < 2 else (sset[2], sset[3])
                        nc.gpsimd.dma_scatter_add(
                            out_ap=own[:].rearrange("p (g c) -> p g c", c=4),
                            in_ap=src[:, : nt * 4].rearrange(
                                "p (t c) -> p t c", c=4),
                            idxs_ap=isc[:],
                            num_idxs=sz,
                            num_idxs_reg=sz,
                            elem_size=4,
                            sbuf_tokens_per_rank=P,
                            parity_reg=0,
                            out_ap_other=oth[:].rearrange("p (g c) -> p g c", c=4),
                            queue_num=gctr % 4)
                    e0b += sz
                    gctr += 1

            # ---- merge surface sets, dump raw surfaces ----
            for i in range(4):
                acc = surf[0][i]
                for s in range(1, NSETS):
                    nc.vector.tensor_add(acc[:], acc[:], surf[s][i][:])
                nc.sync.dma_start(out=aggs_t[:, i * G4 : (i + 1) * G4], in_=acc[:])
            loop_cm.__exit__(None, None, None)

    nc.finalize()
    return nc


def _tiled(m, w):
    """Node offset within a half -> flat row in the tiled table layout."""
    return (m % P) * w + m // P


def _prep_core(rows, cols, cdm, attr):
    """Reorder one core's edges into bucketed layout.

    Returns gather-idx [NBUCK, GI_PER_B, 2, 128, NI/16] i16,
    scatter-idx [NBUCK, GI_PER_B, 128, NI/16] i16,
    cdm [128, TTOT*4] bf16, attrT [NBUCK, 1, BCAP] bf16.
    """
    idxg = np.zeros((NBUCK, 2, P, GI_PER_B * (NI // 16)), np.int16)
    idxsc = np.zeros((NBUCK, P, GI_PER_B * (NI // 16)), np.int16)
    cdmR = np.zeros((TTOT, P, 4), ml_dtypes.bfloat16)
    attrT = np.zeros((NBUCK, 1, BCAP), ml_dtypes.bfloat16)
    bucket = (rows >= HALF).astype(np.int64) * 2 + (cols >= HALF).astype(np.int64)
    for b in range(NBUCK):
        sel = np.nonzero(bucket == b)[0]
        eb = len(sel)
        assert eb <= BCAP, f"bucket {b} has {eb} edges > cap {BCAP}"
        # order edges by (row block, col) for DRAM locality in both gathers
        sel = sel[np.lexsort((cols[sel], rows[sel] >> 9))]
        ra_f = np.zeros(BCAP, np.int16)         # gather idx into A half-table
        cb_f = np.zeros(BCAP, np.int16)         # gather idx into B half-table
        sc_f = np.zeros(BCAP, np.int16)         # scatter idx (0 = dump)
        cdm_f = np.zeros((BCAP, 4), np.float32)
        at_f = np.zeros(BCAP, np.float32)
        mr = rows[sel] - (HALF if b >= 2 else 0)
        mc = cols[sel] - (HALF if b % 2 else 0)
        ra_f[:eb] = _tiled(mr, WHI if b >= 2 else WLO).astype(np.int16)
        cb_f[:eb] = _tiled(mc, WHI if b % 2 else WLO).astype(np.int16)
        sc_f[:eb] = (mr + 256).astype(np.int16)
        cdm_f[:eb, :3] = cdm[sel]
        at_f[:eb] = attr[sel]
        # token i of each NI batch lives at [i%128, i//128] of the tile group
        bt0 = b * TB
        cdmR[bt0 : bt0 + TB] = cdm_f.reshape(TB, P, 4)
        attrT[b, 0, :] = at_f.astype(ml_dtypes.bfloat16)
        for gi in range(GI_PER_B):
            seg = slice(gi * NI, (gi + 1) * NI)
            csl = slice(gi * (NI // 16), (gi + 1) * (NI // 16))
            idxg[b, 0, :, csl] = _wrap_idx(ra_f[seg])
            idxg[b, 1, :, csl] = _wrap_idx(cb_f[seg])
            idxsc[b, :, csl] = _wrap_idx(sc_f[seg])
    return (idxg, idxsc,
            np.ascontiguousarray(cdmR.transpose(1, 0, 2)).reshape(P, TTOT * 4),
            attrT)


def _decode_aggs(buf):
    """[128, 4*G4] raw surfaces -> [N, 3] f32 partial aggregate."""
    surfs = [buf[:, i * G4 : (i + 1) * G4].reshape(P, G, 4) for i in range(4)]
    agg = np.zeros((N, 3), np.float32)
    for base, hi, s0, s1 in ((0, NLO, surfs[0], surfs[1]),
                             (NLO, N, surfs[2], surfs[3])):
        n = np.arange(base, hi)
        idx = (n - base) + 256
        p = idx & 127
        slot = idx >> 7
        g = slot >> 1
        pair = np.stack([s0, s1])          # [2, 128, G, 4]
        agg[n] = pair[slot & 1, p, g, :3]
    return agg


def kernel(h, coord, edge_index, coord_diff, edge_attr, node_mask, edge_mask,
           W1, b1, W2, b2, W3):
    h = np.asarray(h, np.float32)
    coord = np.asarray(coord, np.float32)
    edge_index = np.asarray(edge_index)
    coord_diff = np.asarray(coord_diff, np.float32)
    edge_attr = np.asarray(edge_attr, np.float32)
    node_mask = np.asarray(node_mask, np.float32)
    edge_mask = np.asarray(edge_mask, np.float32)
    W1 = np.asarray(W1, np.float32)
    b1 = np.asarray(b1, np.float32)
    W2 = np.asarray(W2, np.float32)
    b2 = np.asarray(b2, np.float32)
    W3 = np.asarray(W3, np.float32)

    rows = edge_index[0].astype(np.int32)
    cols = edge_index[1].astype(np.int32)
    cdm = coord_diff * edge_mask  # fold edge mask

    bf = ml_dtypes.bfloat16
    hT = np.ascontiguousarray(h.T).astype(bf)
    w1abT = np.ascontiguousarray(
        np.concatenate([W1[:, :H].T, W1[:, H : 2 * H].T], axis=1)).astype(bf)
    w1c = np.ascontiguousarray(W1[:, 2 * H][None, :]).astype(bf)
    w2T = np.ascontiguousarray(W2.T).astype(bf)
    w3Ts = np.ascontiguousarray(W3.T / 100.0).astype(bf)
    b1c = np.ascontiguousarray(b1[:, None]).astype(np.float32)
    b2c = np.ascontiguousarray(b2[:, None]).astype(np.float32)

    if "nc" not in _nc_cache:
        _nc_cache["nc"] = _build_program()
    nc = _nc_cache["nc"]

    in_maps = []
    for c in range(NCORES):
        sl = slice(c * ECORE, (c + 1) * ECORE)
        idxg, idxsc, cdmR, attrT = _prep_core(
            rows[sl], cols[sl], cdm[sl], edge_attr[sl, 0])
        in_maps.append({
            "hT": hT, "w1abT": w1abT, "w1c": w1c, "w2T": w2T, "w3Ts": w3Ts,
            "b1c": b1c, "b2c": b2c,
            "idxg": idxg, "idxsc": idxsc, "cdm": cdmR, "attrT": attrT,
        })

    res = run_bass_kernel_spmd(nc, in_maps, list(range(NCORES))).results
    agg = np.zeros((N, 3), np.float32)
    for c in range(NCORES):
        agg += _decode_aggs(res[c]["aggs"])
    return (coord + agg) * node_mask
